# revision 1
# baseline (speedup 1.0000x reference)
"""Trainium2 Bass kernel for nn_CNNT_enhanced_denoising_runtime_53704271069472.

Computes, distributed across 8 NeuronCores:
    q/k/v = conv3x3(x, w?, b?)          (image-sharded: B*T=128 imgs, 16/core)
    att   = causal-softmax(q @ k^T / sqrt(D)) per (batch, head)
    y     = att @ v                      (head-sharded: 16 (b,head) pairs, 2/core)
    out   = conv3x3(y, wo, bo)           (image-sharded)

Three SPMD launches with host-side resharding between them. Convs are done as
matmuls over K = (3 kx-taps x 16 ch [+ ones bias row]) x 2 ky-rows = 97(+48)
against kx-pre-shifted zero-padded image planes built on the host; the 3x3
kernel's third ky row is a second accumulating matmul with an AP row offset.
Compute dtype bf16 (fp32 PSUM accumulation).
"""
import sys
import numpy as np

sys.path.insert(0, "/opt/trn_rl_repo")

import ml_dtypes  # noqa: E402
import concourse.bacc as bacc  # noqa: E402
import concourse.tile as tile  # noqa: E402
import concourse.bass as bass  # noqa: E402
from concourse import mybir, bass_utils  # noqa: E402

BF16 = mybir.dt.bfloat16
F32 = mybir.dt.float32
NPBF16 = ml_dtypes.bfloat16

B, T, C, H, W, O = 2, 64, 16, 128, 128, 16
HP, WP = H + 2, W + 2
HW = H * W
NH, HC = 8, 2
D = HC * HW
SCALE = float(1.0 / np.sqrt(np.float32(D)))
NCORES = 8
IMGS = B * T
IPC = IMGS // NCORES  # images per core
NPL = 98  # plane rows: 48 (ky0 kx-taps) + ones + 48 (ky1) + pad

_BUILD_CACHE = {}


# ---------------- device programs ----------------

def _build_l1():
    nc = bacc.Bacc("TRN2", target_bir_lowering=False, debug=False)
    planes = nc.dram_tensor("planes", (IPC, NPL, HP * WP), BF16, kind="ExternalInput")
    lhsT0 = nc.dram_tensor("lhsT0", (97, 48), BF16, kind="ExternalInput")
    lhsT1 = nc.dram_tensor("lhsT1", (48, 48), BF16, kind="ExternalInput")
    qkv = nc.dram_tensor("qkv_out", (IPC, 128, 8192), BF16, kind="ExternalOutput")

    with tile.TileContext(nc) as tc:
        with tc.tile_pool(name="w", bufs=1) as wpool, \
             tc.tile_pool(name="pl", bufs=3) as plpool, \
             tc.tile_pool(name="st", bufs=3) as stpool, \
             tc.tile_pool(name="ps", bufs=4, space="PSUM") as pspool:
            w0 = wpool.tile([97, 48], BF16, tag="w0")
            w1 = wpool.tile([48, 48], BF16, tag="w1")
            nc.sync.dma_start(w0[:], lhsT0.ap())
            nc.sync.dma_start(w1[:], lhsT1.ap())

            def rhs_view(pt, nrows, blk, ky):
                base = (blk * 4 + ky) * WP
                return pt[0:nrows, base:base + 4 * WP].rearrange(
                    "p (h w) -> p h w", w=WP)[:, :, 0:W]

            for img in range(IPC):
                pt = plpool.tile([NPL, HP * WP], BF16)
                nc.scalar.dma_start(pt[:], planes.ap()[img])
                stage = stpool.tile([128, 8192], BF16)
                for q4 in range(8):
                    ps = pspool.tile([128, 1024], F32)
                    for sub in range(2):
                        for half in range(2):
                            blk = q4 * 4 + sub * 2 + half
                            psv = ps[half * 64:half * 64 + 48,
                                     sub * 512:sub * 512 + 512]
                            nc.tensor.matmul(psv, w0[:], rhs_view(pt, 97, blk, 0),
                                             start=True, stop=False,
                                             tile_position=(0, half * 64))
                            nc.tensor.matmul(psv, w1[:], rhs_view(pt, 48, blk, 2),
                                             start=False, stop=True,
                                             tile_position=(0, half * 64))
                    nc.vector.tensor_copy(stage[:, q4 * 1024:(q4 + 1) * 1024], ps[:])
                nc.sync.dma_start(qkv.ap()[img], stage[:])
    nc.compile()
    return nc


def _build_l2():
    nc = bacc.Bacc("TRN2", target_bir_lowering=False, debug=False)
    qks = nc.dram_tensor("qks", (8, T, HW), BF16, kind="ExternalInput")
    vs = nc.dram_tensor("vs", (4, T, HW), BF16, kind="ExternalInput")
    mask = nc.dram_tensor("mask", (T, T), F32, kind="ExternalInput")
    ident = nc.dram_tensor("ident", (T, T), BF16, kind="ExternalInput")
    ys = nc.dram_tensor("ys", (2, 128, HW), BF16, kind="ExternalOutput")

    with tile.TileContext(nc) as tc:
        with tc.tile_pool(name="cst", bufs=1) as cst, \
             tc.tile_pool(name="qk", bufs=6) as qkpool, \
             tc.tile_pool(name="sm", bufs=2) as smpool, \
             tc.tile_pool(name="v", bufs=8) as vpool, \
             tc.tile_pool(name="yst", bufs=2) as ypool, \
             tc.tile_pool(name="pst", bufs=1, space="PSUM") as pstpool, \
             tc.tile_pool(name="psy", bufs=3, space="PSUM") as psypool, \
             tc.tile_pool(name="psl", bufs=1, space="PSUM") as pslpool:
            mask_t = cst.tile([T, T], F32, tag="mask")
            nc.sync.dma_start(mask_t[:], mask.ap())
            id_t = cst.tile([T, T], BF16, tag="ident")
            nc.sync.dma_start(id_t[:], ident.ap())

            lg_ps = [pslpool.tile([128, 128], F32, tag=f"lg{h}", name=f"lg{h}")
                     for h in range(2)]
            for blk in range(128):
                qkt = qkpool.tile([128, 512], BF16, tag="qkT")
                src = qks.ap()[:, :, blk * 128:(blk + 1) * 128].rearrange("c t p -> (c t) p")
                nc.sync.dma_start_transpose(qkt[:], src)
                for h in range(2):
                    nc.tensor.matmul(lg_ps[h][:],
                                     qkt[:, h * 128:(h + 1) * 128],
                                     qkt[:, 256 + h * 128:256 + (h + 1) * 128],
                                     start=(blk == 0), stop=(blk == 127))

            attTs = []
            for h in range(2):
                lg = smpool.tile([T, T], F32, tag="lg")
                nc.vector.tensor_copy(lg[:], lg_ps[h][0:64, 0:64])
                nc.vector.tensor_add(lg[:], lg[:], lg_ps[h][64:128, 64:128])
                nc.vector.tensor_scalar(lg[:], lg[:], SCALE, None,
                                        op0=mybir.AluOpType.mult)
                nc.vector.tensor_add(lg[:], lg[:], mask_t[:])
                mx = smpool.tile([T, 1], F32, tag="mx")
                nc.vector.reduce_max(mx[:], lg[:], axis=mybir.AxisListType.X, negate=True)
                e = smpool.tile([T, T], F32, tag="e")
                sm_acc = smpool.tile([T, 1], F32, tag="smacc")
                nc.scalar.activation(e[:], lg[:], mybir.ActivationFunctionType.Exp,
                                     bias=mx[:], scale=1.0, accum_out=sm_acc[:])
                rc = smpool.tile([T, 1], F32, tag="rc")
                nc.vector.reciprocal(rc[:], sm_acc[:])
                att = smpool.tile([T, T], BF16, tag="att")
                nc.vector.tensor_scalar(att[:], e[:], rc[:], None,
                                        op0=mybir.AluOpType.mult)
                ps_t = pstpool.tile([T, T], BF16, tag="pst")
                nc.tensor.transpose(ps_t[:], att[:], id_t[:])
                attT = smpool.tile([128, T], BF16, tag=f"attT{h}", name=f"attT{h}")
                nc.vector.tensor_copy(attT[0:64, :], ps_t[:])
                nc.vector.tensor_copy(attT[64:128, :], ps_t[:])
                attTs.append(attT)

            for p in range(2):
                yst = ypool.tile([128, HW], BF16, tag="yst")
                for vb in range(4):
                    vt = vpool.tile([128, 4096], BF16, tag="vt")
                    src_v = vs.ap()[2 * p:2 * p + 2, :, vb * 4096:(vb + 1) * 4096]
                    nc.scalar.dma_start(vt[:], src_v.rearrange("c t p -> (c t) p"))
                    for ci in range(2):
                        attT = attTs[p]
                        for j in range(8):
                            ps_y = psypool.tile([T, 512], F32, tag="psy")
                            nc.tensor.matmul(ps_y[:], attT[ci * 64:ci * 64 + 64, :],
                                             vt[ci * 64:ci * 64 + 64, j * 512:(j + 1) * 512],
                                             start=True, stop=True)
                            col = vb * 4096 + j * 512
                            nc.vector.tensor_copy(
                                yst[ci * 64:ci * 64 + 64, col:col + 512], ps_y[:])
                nc.sync.dma_start(ys.ap()[p], yst[:])
    nc.compile()
    return nc


def _build_l3():
    nc = bacc.Bacc("TRN2", target_bir_lowering=False, debug=False)
    planes = nc.dram_tensor("planes", (IPC, NPL, HP * WP), BF16, kind="ExternalInput")
    lhsT0 = nc.dram_tensor("lhsT0", (97, 16), BF16, kind="ExternalInput")
    lhsT1 = nc.dram_tensor("lhsT1", (48, 16), BF16, kind="ExternalInput")
    out = nc.dram_tensor("out", (IPC, 80, 8192), F32, kind="ExternalOutput")

    with tile.TileContext(nc) as tc:
        with tc.tile_pool(name="w", bufs=1) as wpool, \
             tc.tile_pool(name="pl", bufs=3) as plpool, \
             tc.tile_pool(name="st", bufs=3) as stpool, \
             tc.tile_pool(name="ps", bufs=4, space="PSUM") as pspool:
            w0 = wpool.tile([97, 16], BF16, tag="w0")
            w1 = wpool.tile([48, 16], BF16, tag="w1")
            nc.sync.dma_start(w0[:], lhsT0.ap())
            nc.sync.dma_start(w1[:], lhsT1.ap())

            def rhs_view(pt, nrows, blk, ky):
                base = (blk * 4 + ky) * WP
                return pt[0:nrows, base:base + 4 * WP].rearrange(
                    "p (h w) -> p h w", w=WP)[:, :, 0:W]

            for img in range(IPC):
                pt = plpool.tile([NPL, HP * WP], BF16)
                nc.scalar.dma_start(pt[:], planes.ap()[img])
                stage = stpool.tile([128, 8192], F32)
                for q4 in range(8):
                    ps = pspool.tile([128, 1024], F32)
                    for sub in range(2):
                        for half in range(2):
                            blk = q4 * 4 + sub * 2 + half
                            po = half * 64
                            psv = ps[po:po + 16, sub * 512:sub * 512 + 512]
                            nc.tensor.matmul(psv, w0[:], rhs_view(pt, 97, blk, 0),
                                             start=True, stop=False,
                                             tile_position=(0, po))
                            nc.tensor.matmul(psv, w1[:], rhs_view(pt, 48, blk, 2),
                                             start=False, stop=True,
                                             tile_position=(0, po))
                    nc.vector.tensor_copy(stage[0:80, q4 * 1024:(q4 + 1) * 1024],
                                          ps[0:80, :])
                nc.sync.dma_start(out.ap()[img], stage[0:80, :])
    nc.compile()
    return nc


def _get(name):
    if name not in _BUILD_CACHE:
        _BUILD_CACHE[name] = {"l1": _build_l1, "l2": _build_l2, "l3": _build_l3}[name]()
    return _BUILD_CACHE[name]


# ---------------- host-side packing ----------------

def _build_planes(imgs_chw):
    """imgs_chw: [N, 16, H, W] float32-like -> [N, 98, HP*WP] bf16."""
    N = imgs_chw.shape[0]
    xpad = np.zeros((N, C, HP, WP), np.float32)
    xpad[:, :, 1:H + 1, 1:W + 1] = imgs_chw.astype(np.float32)
    flat = xpad.reshape(N, C, HP * WP)
    p = np.zeros((N, NPL, HP * WP), np.float32)
    p[:, 0:16] = flat
    p[:, 16:32, :-1] = flat[:, :, 1:]
    p[:, 32:48, :-2] = flat[:, :, 2:]
    p[:, 48] = 1.0
    p[:, 49:97, :-WP] = p[:, 0:48, WP:]
    return p.astype(NPBF16)


def _build_lhsT(ws, bs):
    """ws: list of [O,C,3,3]; bs: list of [O] -> lhsT0 [97, 16*len], lhsT1 [48, 16*len]."""
    n = len(ws)
    m = np.zeros((3, 49, 16 * n), np.float32)
    for j, (w, b) in enumerate(zip(ws, bs)):
        for ky in range(3):
            for kx in range(3):
                m[ky, kx * 16:(kx + 1) * 16, j * 16:(j + 1) * 16] = w[:, :, ky, kx].T
        m[1, 48, j * 16:(j + 1) * 16] = b
    l0 = np.zeros((97, 16 * n), np.float32)
    l0[0:48] = m[0][0:48]
    l0[48] = m[1][48]
    l0[49:97] = m[1][0:48]
    return l0.astype(NPBF16), m[2][0:48].astype(NPBF16)


def _unpack_qkv(qkv_out):
    """[N,128,8192] bf16 -> q,k,v each [N,16,HW].

    blk = q4*4 + sub*2 + half lives at stage rows half*64(+48), col q4*1024+sub*512."""
    N = qkv_out.shape[0]
    s = qkv_out.reshape(N, 128, 8, 2, 512)       # [N, p, q4, sub, 512]
    out = np.empty((N, 48, 8, 2, 2, 512), qkv_out.dtype)  # [N, c, q4, sub, half, 512]
    out[..., 0, :] = s[:, 0:48]
    out[..., 1, :] = s[:, 64:112]
    out = out.reshape(N, 48, HW)
    return out[:, 0:16], out[:, 16:32], out[:, 32:48]


def _unpack_l3(o):
    """[N,80,8192] f32 -> [N,16,HW].

    blk = q4*4 + sub*2 + half lives at row (half*64)+c, col q4*1024 + sub*512
    (rows 16-63 are junk from the spanning psum copy)."""
    N = o.shape[0]
    s = o.reshape(N, 80, 8, 2, 512)      # [N, row, q4, sub, 512]
    out = np.empty((N, 16, 32, 512), o.dtype)
    for q4 in range(8):
        for sub in range(2):
            for half in range(2):
                blk = q4 * 4 + sub * 2 + half
                out[:, :, blk] = s[:, half * 64:half * 64 + 16, q4, sub]
    return np.ascontiguousarray(out).reshape(N, 16, HW)


# ---------------- top level ----------------

def kernel(x, wq, bq, wk, bk, wv, bv, wo, bo):
    x, wq, bq, wk, bk, wv, bv, wo, bo = (
        np.asarray(a, np.float32) for a in (x, wq, bq, wk, bk, wv, bv, wo, bo))
    ximg = x.reshape(IMGS, C, H, W)
    cores = list(range(NCORES))

    # ---- L1: q/k/v convs, image-sharded
    l0, l1 = _build_lhsT([wq, wk, wv], [bq, bk, bv])
    in_maps = [{"planes": _build_planes(ximg[c * IPC:(c + 1) * IPC]),
                "lhsT0": l0, "lhsT1": l1} for c in cores]
    res1 = bass_utils.run_bass_kernel_spmd(_get("l1"), in_maps, core_ids=cores)

    # assemble channel-major [B, 16, T, HW] bf16
    q_all = np.empty((B, 16, T, HW), NPBF16)
    k_all = np.empty_like(q_all)
    v_all = np.empty_like(q_all)
    for c in cores:
        q, k, v = _unpack_qkv(res1.results[c]["qkv_out"])
        b0 = (c * IPC) // T
        t0 = (c * IPC) % T
        q_all[b0, :, t0:t0 + IPC] = q.transpose(1, 0, 2)
        k_all[b0, :, t0:t0 + IPC] = k.transpose(1, 0, 2)
        v_all[b0, :, t0:t0 + IPC] = v.transpose(1, 0, 2)

    # ---- L2: attention, head-sharded (2 heads = 4 channels per core)
    mask = np.triu(np.full((T, T), -30000.0, np.float32), 1)
    ident = np.eye(T, dtype=NPBF16)
    in_maps = []
    for c in cores:
        b, g = c // 4, c % 4
        sl = slice(4 * g, 4 * g + 4)
        qks = np.concatenate([q_all[b, sl], k_all[b, sl]], axis=0)
        in_maps.append({"qks": np.ascontiguousarray(qks),
                        "vs": np.ascontiguousarray(v_all[b, sl]),
                        "mask": mask, "ident": ident})
    res2 = bass_utils.run_bass_kernel_spmd(_get("l2"), in_maps, core_ids=cores)

    y_all = np.empty((B, 16, T, HW), NPBF16)
    for c in cores:
        b, g = c // 4, c % 4
        ys = res2.results[c]["ys"]
        for p in range(2):
            y_all[b, 4 * g + 2 * p] = ys[p, 0:64]
            y_all[b, 4 * g + 2 * p + 1] = ys[p, 64:128]

    # ---- L3: output conv, image-sharded
    yimg = y_all.astype(np.float32).transpose(0, 2, 1, 3).reshape(IMGS, 16, H, W)
    l0o, l1o = _build_lhsT([wo], [bo])
    in_maps = [{"planes": _build_planes(yimg[c * IPC:(c + 1) * IPC]),
                "lhsT0": l0o, "lhsT1": l1o} for c in cores]
    res3 = bass_utils.run_bass_kernel_spmd(_get("l3"), in_maps, core_ids=cores)

    out = np.concatenate([_unpack_l3(res3.results[c]["out"]) for c in cores])
    return np.ascontiguousarray(out.reshape(B, T, O, H, W))



# revision 2
# speedup vs baseline: 2.1858x; 2.1858x over previous
"""Trainium2 Bass kernel for nn_CNNT_enhanced_denoising_runtime_53704271069472.

Computes, distributed across 8 NeuronCores:
    q/k/v = conv3x3(x, w?, b?)          (image-sharded: B*T=128 imgs, 16/core)
    att   = causal-softmax(q @ k^T / sqrt(D)) per (batch, head)
    y     = att @ v                      (head-sharded: 16 (b,head) pairs, 2/core)
    out   = conv3x3(y, wo, bo)           (image-sharded)

Three SPMD launches with host-side resharding between them.

Convs use the matmul orientation lhsT = image planes (stationary, M = 128
pixels of one image row), rhs = weights (moving, N = out-channels). Planes
are 49 rows: 16 channels x 3 kx-shifts + a ones row (bias); the 3 ky taps
are 3 PSUM-accumulating matmuls with a free-dim row offset on the planes.

Attention per (batch, head) pair: logits = 256 K=128 chunk matmuls
(lhsT = qT chunk [128d, 64t], rhs = kT chunk [128d, 64s]) accumulated in
PSUM; row softmax; att transposed on the PE and placed block-diagonally in
a [128,128] lhsT so att@v contracts both head-channels in single K=128
matmuls. Host pre-transposes q/k into chunk-major layout (no device
transpose DMAs) and pre-scales q by 1/sqrt(D).

Compute dtype bf16 (fp32 PSUM accumulation).
"""
import sys
import numpy as np

sys.path.insert(0, "/opt/trn_rl_repo")

import ml_dtypes  # noqa: E402
import concourse.bacc as bacc  # noqa: E402
import concourse.tile as tile  # noqa: E402
import concourse.bass as bass  # noqa: E402
from concourse import mybir, bass_utils  # noqa: E402

BF16 = mybir.dt.bfloat16
F32 = mybir.dt.float32
NPBF16 = ml_dtypes.bfloat16

B, T, C, H, W, O = 2, 64, 16, 128, 128, 16
HP, WP = H + 2, W + 2
HW = H * W
NH, HC = 8, 2
D = HC * HW
SCALE = float(1.0 / np.sqrt(np.float32(D)))
NCORES = 8
IMGS = B * T
IPC = IMGS // NCORES  # images per core
NPL = 49  # plane rows: 16 ch x 3 kx shifts + ones row

_BUILD_CACHE = {}


# ---------------- device programs ----------------

def _build_conv(nout):
    """Conv launch: image planes in, [128 x, 128 y, nout ch] out per image."""
    nc = bacc.Bacc("TRN2", target_bir_lowering=False, debug=False)
    planes = nc.dram_tensor("planes", (IPC, NPL, HP * WP), BF16, kind="ExternalInput")
    w0 = nc.dram_tensor("w0", (48, nout), BF16, kind="ExternalInput")
    w1 = nc.dram_tensor("w1", (49, nout), BF16, kind="ExternalInput")
    w2 = nc.dram_tensor("w2", (48, nout), BF16, kind="ExternalInput")
    out = nc.dram_tensor("out", (IPC, 128, 128 * nout), BF16, kind="ExternalOutput")

    grp = 480 // nout          # y-rows per PSUM bank
    ngrp = (128 + grp - 1) // grp

    with tile.TileContext(nc) as tc:
        with tc.tile_pool(name="w", bufs=1) as wpool, \
             tc.tile_pool(name="pl", bufs=3) as plpool, \
             tc.tile_pool(name="st", bufs=3) as stpool, \
             tc.tile_pool(name="ps", bufs=6, space="PSUM") as pspool:
            wt0 = wpool.tile([48, nout], BF16, tag="w0")
            wt1 = wpool.tile([49, nout], BF16, tag="w1")
            wt2 = wpool.tile([48, nout], BF16, tag="w2")
            nc.sync.dma_start(wt0[:], w0.ap())
            nc.sync.dma_start(wt1[:], w1.ap())
            nc.sync.dma_start(wt2[:], w2.ap())
            wts = [wt0, wt1, wt2]

            for img in range(IPC):
                pt = plpool.tile([NPL, HP * WP], BF16)
                nc.sync.dma_start(pt[:], planes.ap()[img])
                stage = stpool.tile([128, 128 * nout], BF16)
                for g in range(ngrp):
                    n = min(grp, 128 - g * grp)
                    ps = pspool.tile([128, 512], F32)
                    for j in range(n):
                        y = g * grp + j
                        pv = ps[:, j * nout:(j + 1) * nout]
                        for ky in range(3):
                            rows = 49 if ky == 1 else 48
                            base = (y + ky) * WP
                            nc.tensor.matmul(pv, pt[0:rows, base:base + 128],
                                             wts[ky][:],
                                             start=(ky == 0), stop=(ky == 2))
                    eng = nc.vector if g % 2 == 0 else nc.scalar
                    dst = stage[:, g * grp * nout:(g * grp + n) * nout]
                    if eng is nc.vector:
                        eng.tensor_copy(dst, ps[:, 0:n * nout])
                    else:
                        eng.copy(dst, ps[:, 0:n * nout])
                nc.gpsimd.dma_start(out.ap()[img], stage[:])
    nc.compile()
    return nc


def _build_l1():
    return _build_conv(48)


def _build_l3():
    return _build_conv(16)


def _build_l2():
    nc = bacc.Bacc("TRN2", target_bir_lowering=False, debug=False)
    qT = nc.dram_tensor("qT", (2, 128, D // 2), BF16, kind="ExternalInput")
    kT = nc.dram_tensor("kT", (2, 128, D // 2), BF16, kind="ExternalInput")
    vs = nc.dram_tensor("vs", (2, 128, HW), BF16, kind="ExternalInput")
    mask = nc.dram_tensor("mask", (T, T), F32, kind="ExternalInput")
    ident = nc.dram_tensor("ident", (T, T), BF16, kind="ExternalInput")
    ys = nc.dram_tensor("ys", (2, 128, HW), BF16, kind="ExternalOutput")

    NCH = 256       # K=128 contraction chunks per pair
    GCH = 64        # chunks per DMA group ([128, 4096])

    with tile.TileContext(nc) as tc:
        with tc.tile_pool(name="cst", bufs=1) as cst, \
             tc.tile_pool(name="qk", bufs=6) as qkpool, \
             tc.tile_pool(name="sm", bufs=2) as smpool, \
             tc.tile_pool(name="v", bufs=2) as vpool, \
             tc.tile_pool(name="yst", bufs=2) as ypool, \
             tc.tile_pool(name="pslg", bufs=2, space="PSUM") as lgpool, \
             tc.tile_pool(name="pst", bufs=1, space="PSUM") as pstpool, \
             tc.tile_pool(name="psy", bufs=4, space="PSUM") as psypool:
            mask_t = cst.tile([T, T], F32, tag="mask")
            nc.sync.dma_start(mask_t[:], mask.ap())
            id_t = cst.tile([T, T], BF16, tag="ident")
            nc.sync.dma_start(id_t[:], ident.ap())
            bds = []
            for p in range(2):
                bd = cst.tile([128, 128], BF16, tag=f"bd{p}")
                nc.vector.memset(bd[:], 0.0)
                bds.append(bd)

            for p in range(2):
                # ---- logits: 256 chunk matmuls, K=128 each
                lg_ps = lgpool.tile([T, 512], F32)
                for g in range(NCH // GCH):
                    qt = qkpool.tile([128, GCH * T], BF16, tag="qt")
                    nc.sync.dma_start(qt[:], qT.ap()[p, :, g * GCH * T:(g + 1) * GCH * T])
                    kt = qkpool.tile([128, GCH * T], BF16, tag="kt")
                    nc.sync.dma_start(kt[:], kT.ap()[p, :, g * GCH * T:(g + 1) * GCH * T])
                    for cc in range(GCH):
                        ci = g * GCH + cc
                        nc.tensor.matmul(lg_ps[:, 0:T],
                                         qt[:, cc * T:(cc + 1) * T],
                                         kt[:, cc * T:(cc + 1) * T],
                                         start=(ci == 0), stop=(ci == NCH - 1))

                # ---- softmax over s (free dim); q was pre-scaled by 1/sqrt(D)
                lg = smpool.tile([T, T], F32, tag="lg")
                nc.vector.tensor_add(lg[:], lg_ps[:, 0:T], mask_t[:])
                mx = smpool.tile([T, 1], F32, tag="mx")
                nc.vector.reduce_max(mx[:], lg[:], axis=mybir.AxisListType.X, negate=True)
                e = smpool.tile([T, T], F32, tag="e")
                sm_acc = smpool.tile([T, 1], F32, tag="smacc")
                nc.scalar.activation(e[:], lg[:], mybir.ActivationFunctionType.Exp,
                                     bias=mx[:], scale=1.0, accum_out=sm_acc[:])
                rc = smpool.tile([T, 1], F32, tag="rc")
                nc.vector.reciprocal(rc[:], sm_acc[:])
                att = smpool.tile([T, T], BF16, tag="att")
                nc.vector.tensor_scalar(att[:], e[:], rc[:], None,
                                        op0=mybir.AluOpType.mult)

                # ---- attT placed block-diagonally for K=128 att@v
                ps_t = pstpool.tile([T, 1024], BF16)
                nc.tensor.transpose(ps_t[:, 0:T], att[:], id_t[:])
                nc.vector.tensor_copy(bds[p][0:T, 0:T], ps_t[:, 0:T])
                nc.vector.tensor_copy(bds[p][T:128, T:128], ps_t[:, 0:T])

                # ---- y = att @ v, both channels per matmul
                for h in range(2):
                    vt = vpool.tile([128, HW // 2], BF16)
                    nc.sync.dma_start(vt[:], vs.ap()[p, :, h * (HW // 2):(h + 1) * (HW // 2)])
                    yst = ypool.tile([128, HW // 2], BF16)
                    for j in range(16):
                        psy = psypool.tile([128, 512], F32)
                        nc.tensor.matmul(psy[:], bds[p][:],
                                         vt[:, j * 512:(j + 1) * 512],
                                         start=True, stop=True)
                        dst = yst[:, j * 512:(j + 1) * 512]
                        if j % 2 == 0:
                            nc.vector.tensor_copy(dst, psy[:])
                        else:
                            nc.scalar.copy(dst, psy[:])
                    nc.gpsimd.dma_start(ys.ap()[p, :, h * (HW // 2):(h + 1) * (HW // 2)],
                                        yst[:])
    nc.compile()
    return nc


def _get(name):
    if name not in _BUILD_CACHE:
        _BUILD_CACHE[name] = {"l1": _build_l1, "l2": _build_l2, "l3": _build_l3}[name]()
    return _BUILD_CACHE[name]


# ---------------- host-side packing ----------------

def _build_planes(imgs_chw):
    """imgs_chw: [N, 16, H, W] float32-like -> [N, 49, HP*WP] bf16.

    Row r = kx*16 + c holds xpad[c, :, :] shifted left by kx; row 48 is ones."""
    N = imgs_chw.shape[0]
    xpad = np.zeros((N, C, HP, WP), np.float32)
    xpad[:, :, 1:H + 1, 1:W + 1] = np.asarray(imgs_chw, np.float32)
    flat = xpad.reshape(N, C, HP * WP)
    p = np.empty((N, NPL, HP * WP), np.float32)
    p[:, 0:16] = flat
    p[:, 16:32, :-1] = flat[:, :, 1:]
    p[:, 16:32, -1] = 0.0
    p[:, 32:48, :-2] = flat[:, :, 2:]
    p[:, 32:48, -2:] = 0.0
    p[:, 48] = 1.0
    return p.astype(NPBF16)


def _build_w(ws, bs):
    """ws: list of [O,C,3,3]; bs: list of [O] -> (w0 [48,16n], w1 [49,16n], w2 [48,16n]).

    Row r = kx*16 + c matches the plane layout; bias rides the ones row in w1."""
    n = len(ws)
    out = []
    for ky in range(3):
        m = np.zeros((49, 16 * n), np.float32)
        for j, w in enumerate(ws):
            for kx in range(3):
                m[kx * 16:(kx + 1) * 16, j * 16:(j + 1) * 16] = w[:, :, ky, kx].T
        if ky == 1:
            for j, b in enumerate(bs):
                m[48, j * 16:(j + 1) * 16] = b
            out.append(m.astype(NPBF16))
        else:
            out.append(m[0:48].astype(NPBF16))
    return out


def _unpack_conv(o, nout):
    """[N, 128 x, 128*nout (y-major, ch-minor)] -> [N, nout, HW]."""
    N = o.shape[0]
    return np.ascontiguousarray(
        o.reshape(N, 128, 128, nout).transpose(0, 3, 2, 1)).reshape(N, nout, HW)


def _pack_chunkT(a):
    """[2 ch, T, HW] -> chunk-major transposed [128, 256*T]."""
    a = np.ascontiguousarray(a.transpose(0, 2, 1)).reshape(D, T)
    return np.ascontiguousarray(a.reshape(256, 128, T).transpose(1, 0, 2)).reshape(128, 256 * T)


# ---------------- top level ----------------

def kernel(x, wq, bq, wk, bk, wv, bv, wo, bo):
    x, wq, bq, wk, bk, wv, bv, wo, bo = (
        np.asarray(a, np.float32) for a in (x, wq, bq, wk, bk, wv, bv, wo, bo))
    ximg = x.reshape(IMGS, C, H, W)
    cores = list(range(NCORES))

    # ---- L1: q/k/v convs, image-sharded
    w0, w1, w2 = _build_w([wq, wk, wv], [bq, bk, bv])
    in_maps = [{"planes": _build_planes(ximg[c * IPC:(c + 1) * IPC]),
                "w0": w0, "w1": w1, "w2": w2} for c in cores]
    res1 = bass_utils.run_bass_kernel_spmd(_get("l1"), in_maps, core_ids=cores)

    # assemble [B, 48, T, HW] bf16 (channel-major)
    qkv_all = np.empty((B, 48, T, HW), NPBF16)
    for c in cores:
        qkv = _unpack_conv(res1.results[c]["out"], 48)  # [IPC, 48, HW]
        b0 = (c * IPC) // T
        t0 = (c * IPC) % T
        qkv_all[b0, :, t0:t0 + IPC] = qkv.transpose(1, 0, 2)
    q_all = (qkv_all[:, 0:16].astype(np.float32) * SCALE).astype(NPBF16)
    k_all = qkv_all[:, 16:32]
    v_all = qkv_all[:, 32:48]

    # ---- L2: attention, head-sharded (2 (b,head) pairs per core)
    mask = np.triu(np.full((T, T), -30000.0, np.float32), 1)
    ident = np.eye(T, dtype=NPBF16)
    in_maps = []
    for c in cores:
        qTs = np.empty((2, 128, 256 * T), NPBF16)
        kTs = np.empty((2, 128, 256 * T), NPBF16)
        vss = np.empty((2, 128, HW), NPBF16)
        for p in range(2):
            pi = 2 * c + p
            b, h = divmod(pi, NH)
            sl = slice(2 * h, 2 * h + 2)
            qTs[p] = _pack_chunkT(q_all[b, sl])
            kTs[p] = _pack_chunkT(k_all[b, sl])
            vss[p] = v_all[b, sl].reshape(128, HW)
        in_maps.append({"qT": qTs, "kT": kTs, "vs": vss, "mask": mask, "ident": ident})
    res2 = bass_utils.run_bass_kernel_spmd(_get("l2"), in_maps, core_ids=cores)

    y_all = np.empty((B, 16, T, HW), NPBF16)
    for c in cores:
        ys = res2.results[c]["ys"]  # [2, 128 (c,t), HW]
        for p in range(2):
            pi = 2 * c + p
            b, h = divmod(pi, NH)
            y_all[b, 2 * h:2 * h + 2] = ys[p].reshape(2, T, HW)

    # ---- L3: output conv, image-sharded
    yimg = np.ascontiguousarray(
        y_all.astype(np.float32).transpose(0, 2, 1, 3)).reshape(IMGS, 16, H, W)
    w0o, w1o, w2o = _build_w([wo], [bo])
    in_maps = [{"planes": _build_planes(yimg[c * IPC:(c + 1) * IPC]),
                "w0": w0o, "w1": w1o, "w2": w2o} for c in cores]
    res3 = bass_utils.run_bass_kernel_spmd(_get("l3"), in_maps, core_ids=cores)

    out = np.concatenate([_unpack_conv(res3.results[c]["out"], 16).astype(np.float32)
                          for c in cores])
    return np.ascontiguousarray(out.reshape(B, T, O, H, W))


# revision 29
# speedup vs baseline: 2.5025x; 1.1449x over previous
"""Trainium2 Bass kernel for nn_CNNT_enhanced_denoising_runtime_53704271069472.

Computes, distributed across 8 NeuronCores:
    q/k/v = conv3x3(x, w?, b?)          (image-sharded: B*T=128 imgs, 16/core)
    att   = causal-softmax(q @ k^T / sqrt(D)) per (batch, head)
    y     = att @ v                      (head-sharded: 16 (b,head) pairs, 2/core)
    out   = conv3x3(y, wo, bo)           (image-sharded)

Three SPMD launches with host-side resharding between them.

Convs use the matmul orientation lhsT = image planes (stationary, M = 128
pixels of one image row), rhs = weights (moving, N = out-channels). L1
planes are 49 rows (16 ch x 3 kx-shifts + ones row for bias); the 3 ky taps
are PSUM-accumulating matmuls with a free-dim row offset. L3 planes carry
only 2 kx copies (33 rows); the kx=2 tap reads the kx=0 rows at +2.

q/k travel as fp8e4m3 (they only feed the softmax logits, where the
relative error is crushed by the tiny logit scale); v/y stay bf16. The
1/sqrt(D) logit scale is folded into the softmax exp (fp8 can't hold
pre-scaled q).

Attention per (batch, head) pair: logits = 256 K=128 chunk matmuls
(lhsT = qT chunk [128d, 64t], rhs = kT chunk [128d, 64s]) accumulated in
PSUM; row softmax; att transposed on the PE and placed block-diagonally in
a [128,128] lhsT so att@v contracts both head-channels in single K=128
matmuls. Host pre-transposes q/k into chunk-major layout.
"""
import sys
import numpy as np

sys.path.insert(0, "/opt/trn_rl_repo")

import ml_dtypes  # noqa: E402
import concourse.bacc as bacc  # noqa: E402
import concourse.tile as tile  # noqa: E402
import concourse.bass as bass  # noqa: E402
from concourse import mybir, bass_utils  # noqa: E402

BF16 = mybir.dt.bfloat16
FP8 = mybir.dt.float8e4
F32 = mybir.dt.float32
NPBF16 = ml_dtypes.bfloat16
NPFP8 = ml_dtypes.float8_e4m3

B, T, C, H, W, O = 2, 64, 16, 128, 128, 16
HP, WP = H + 2, W + 2
HW = H * W
NH, HC = 8, 2
D = HC * HW
SCALE = float(1.0 / np.sqrt(np.float32(D)))
NCORES = 8
IMGS = B * T
IPC = IMGS // NCORES  # images per core

_BUILD_CACHE = {}


# ---------------- device programs ----------------

def _build_l1():
    """q/k/v convs. Out: q,k as fp8 [x, y*32+ch], v as bf16 [x, y*16+ch]."""
    nc = bacc.Bacc("TRN2", target_bir_lowering=False, debug=False)
    planes = nc.dram_tensor("planes", (IPC, 49, HP * WP), BF16, kind="ExternalInput")
    wpk = nc.dram_tensor("wpk", (49, 144), BF16, kind="ExternalInput")
    out_qk = nc.dram_tensor("out_qk", (IPC, 128, 128 * 32), FP8, kind="ExternalOutput")
    out_v = nc.dram_tensor("out_v", (IPC, 128, 128 * 16), BF16, kind="ExternalOutput")

    with tile.TileContext(nc) as tc:
        with tc.tile_pool(name="w", bufs=1) as wpool, \
             tc.tile_pool(name="pl", bufs=3) as plpool, \
             tc.tile_pool(name="pl0", bufs=1) as pl0pool, \
             tc.tile_pool(name="st", bufs=3) as stpool, \
             tc.tile_pool(name="ps", bufs=8, space="PSUM") as pspool:
            # image 0's first planes chunk ships before everything else
            pt_a = pl0pool.tile([49, 66 * WP], BF16, tag="pt0a")
            nc.sync.dma_start(pt_a[:], planes.ap()[0, :, 0:66 * WP])
            wt = wpool.tile([49, 144], BF16, tag="wpk")
            nc.sync.dma_start(wt[:], wpk.ap())
            pt_b = pl0pool.tile([49, 66 * WP], BF16, tag="pt0b")
            nc.sync.dma_start(pt_b[:], planes.ap()[0, :, 64 * WP:130 * WP])

            for img in range(IPC):
                if img == 0:

                    def lhsT(y, ky, rows):
                        if y < 64:
                            return pt_a[0:rows, (y + ky) * WP:(y + ky) * WP + 128]
                        return pt_b[0:rows, (y - 64 + ky) * WP:(y - 64 + ky) * WP + 128]
                else:
                    pt = plpool.tile([49, HP * WP], BF16)
                    nc.sync.dma_start(pt[:], planes.ap()[img])

                    def lhsT(y, ky, rows, pt=pt):
                        return pt[0:rows, (y + ky) * WP:(y + ky) * WP + 128]

                st_qk = stpool.tile([128, 128 * 32], FP8, tag="stqk")
                st_v = stpool.tile([128, 128 * 16], BF16, tag="stv")
                for g in range(13):
                    n = min(10, 128 - g * 10)
                    ps = pspool.tile([128, 512], F32)
                    for j in range(n):
                        y = g * 10 + j
                        pv = ps[:, j * 48:(j + 1) * 48]
                        for ky in range(3):
                            rows = 49 if ky == 1 else 48
                            nc.tensor.matmul(pv, lhsT(y, ky, rows),
                                             wt[0:rows, ky * 48:(ky + 1) * 48],
                                             start=(ky == 0), stop=(ky == 2))
                    src = ps[:, 0:n * 48].rearrange("p (y c) -> p y c", c=48)
                    dqk = st_qk[:, g * 320:(g * 10 + n) * 32].rearrange(
                        "p (y c) -> p y c", c=32)
                    dv = st_v[:, g * 160:(g * 10 + n) * 16].rearrange(
                        "p (y c) -> p y c", c=16)
                    if g % 2 == 0:
                        nc.vector.tensor_copy(dqk, src[:, :, 0:32])
                        nc.scalar.copy(dv, src[:, :, 32:48])
                    else:
                        nc.scalar.copy(dqk, src[:, :, 0:32])
                        nc.vector.tensor_copy(dv, src[:, :, 32:48])
                    if g == 5:  # first 60 rows' columns can ship early
                        nc.gpsimd.dma_start(out_qk.ap()[img, :, 0:1920], st_qk[:, 0:1920])
                        nc.gpsimd.dma_start(out_v.ap()[img, :, 0:960], st_v[:, 0:960])
                    if img == IPC - 1 and g == 9:  # shrink the tail DMA
                        nc.gpsimd.dma_start(out_qk.ap()[img, :, 1920:3200],
                                            st_qk[:, 1920:3200])
                        nc.gpsimd.dma_start(out_v.ap()[img, :, 960:1600],
                                            st_v[:, 960:1600])
                lo_qk, lo_v = (3200, 1600) if img == IPC - 1 else (1920, 960)
                nc.gpsimd.dma_start(out_qk.ap()[img, :, lo_qk:4096], st_qk[:, lo_qk:4096])
                nc.gpsimd.dma_start(out_v.ap()[img, :, lo_v:2048], st_v[:, lo_v:2048])
    nc.compile()
    return nc


N3S3 = 3  # images using 49-row planes / 3 matmuls (DMA-heavy, PE-light)


def _build_l3():
    """Output conv. First N3S3 images use 49-row planes (3 matmuls/row), the
    rest 33-row planes (6 matmuls/row, kx=2 tap reads kx=0 rows at +2) —
    the mix balances the PE and DMA floors."""
    nc = bacc.Bacc("TRN2", target_bir_lowering=False, debug=False)
    planes6 = nc.dram_tensor("planes6", (IPC - N3S3, 33, HP * WP), BF16,
                             kind="ExternalInput")
    planes3 = nc.dram_tensor("planes3", (N3S3, 49, HP * WP), BF16, kind="ExternalInput")
    # packed weights: cols 0:48 = w3 (ky blocks), 48:96 = wa, 96:144 = wb
    wpk = nc.dram_tensor("wpk", (49, 144), BF16, kind="ExternalInput")
    out = nc.dram_tensor("out", (IPC, 128, 128 * 16), BF16, kind="ExternalOutput")

    with tile.TileContext(nc) as tc:
        with tc.tile_pool(name="w", bufs=1) as wpool, \
             tc.tile_pool(name="pl", bufs=3) as plpool, \
             tc.tile_pool(name="pl0", bufs=1) as pl0pool, \
             tc.tile_pool(name="st", bufs=3) as stpool, \
             tc.tile_pool(name="ps", bufs=4, space="PSUM") as pspool:
            pt_a = pl0pool.tile([33, 66 * WP], BF16, tag="pt0a")
            nc.sync.dma_start(pt_a[:], planes6.ap()[0, :, 0:66 * WP])
            wt = wpool.tile([49, 144], BF16, tag="wpk")
            nc.sync.dma_start(wt[:], wpk.ap())
            pt_b = pl0pool.tile([33, 66 * WP], BF16, tag="pt0b")
            nc.sync.dma_start(pt_b[:], planes6.ap()[0, :, 64 * WP:130 * WP])

            for img in range(IPC):
                s3 = img >= IPC - N3S3  # PE-light images at the end
                if img == 0:

                    def win(y, ky, rows, dx=0):
                        if y < 64:
                            b = (y + ky) * WP + dx
                            return pt_a[0:rows, b:b + 128]
                        b = (y - 64 + ky) * WP + dx
                        return pt_b[0:rows, b:b + 128]
                else:
                    pt = plpool.tile([49, HP * WP], BF16)
                    if s3:
                        nc.sync.dma_start(pt[:], planes3.ap()[img - (IPC - N3S3)])
                    else:
                        nc.sync.dma_start(pt[0:33, :], planes6.ap()[img])

                    def win(y, ky, rows, dx=0, pt=pt):
                        b = (y + ky) * WP + dx
                        return pt[0:rows, b:b + 128]

                stage = stpool.tile([128, 128 * 16], BF16)
                # 64 y-rows per 2-bank PSUM tile (32 per bank, dense 16-col)
                for g in range(2):
                    ps = pspool.tile([128, 1024], F32)
                    for j in range(64):
                        y = g * 64 + j
                        pv = ps[:, j * 16:(j + 1) * 16]
                        for ky in range(3):
                            if s3:
                                rows = 49 if ky == 1 else 48
                                nc.tensor.matmul(pv, win(y, ky, rows),
                                                 wt[0:rows, ky * 16:(ky + 1) * 16],
                                                 start=(ky == 0), stop=(ky == 2))
                            else:
                                rows = 33 if ky == 1 else 32
                                nc.tensor.matmul(pv, win(y, ky, rows),
                                                 wt[0:rows, 48 + ky * 16:48 + (ky + 1) * 16],
                                                 start=(ky == 0), stop=False)
                                nc.tensor.matmul(pv, win(y, ky, 16, dx=2),
                                                 wt[0:16, 96 + ky * 16:96 + (ky + 1) * 16],
                                                 start=False, stop=(ky == 2))
                    eng = nc.vector if g % 2 == 0 else nc.scalar
                    dst = stage[:, g * 1024:(g + 1) * 1024]
                    if eng is nc.vector:
                        eng.tensor_copy(dst, ps[:])
                    else:
                        eng.copy(dst, ps[:])
                    if g == 0:
                        nc.gpsimd.dma_start(out.ap()[img, :, 0:1024], stage[:, 0:1024])
                nc.gpsimd.dma_start(out.ap()[img, :, 1024:2048], stage[:, 1024:2048])
    nc.compile()
    return nc


def _build_l2():
    nc = bacc.Bacc("TRN2", target_bir_lowering=False, debug=False)
    qT = nc.dram_tensor("qT", (2, 128, D // 2), FP8, kind="ExternalInput")
    kT = nc.dram_tensor("kT", (2, 128, D // 2), FP8, kind="ExternalInput")
    vs = nc.dram_tensor("vs", (2, 128, HW), BF16, kind="ExternalInput")
    mask = nc.dram_tensor("mask", (T, T), F32, kind="ExternalInput")
    ident = nc.dram_tensor("ident", (T, T), BF16, kind="ExternalInput")
    ys = nc.dram_tensor("ys", (2, 128, HW), BF16, kind="ExternalOutput")

    NCH = 256                        # K=128 contraction chunks per pair
    GROUPS = [16, 48, 64, 64, 64]    # chunks per DMA group (small head chunk)

    with tile.TileContext(nc) as tc:
        with tc.tile_pool(name="cst", bufs=1) as cst, \
             tc.tile_pool(name="qk", bufs=2) as qkpool, \
             tc.tile_pool(name="sm", bufs=2) as smpool, \
             tc.tile_pool(name="v", bufs=2) as vpool, \
             tc.tile_pool(name="yst", bufs=2) as ypool, \
             tc.tile_pool(name="pslg", bufs=2, space="PSUM") as lgpool, \
             tc.tile_pool(name="pst", bufs=1, space="PSUM") as pstpool, \
             tc.tile_pool(name="psy", bufs=4, space="PSUM") as psypool:
            # first q/k chunks ship before everything else so the PE can start
            qt00 = qkpool.tile([128, GROUPS[0] * T], FP8, tag="qt0")
            nc.sync.dma_start(qt00[:], qT.ap()[0, :, 0:GROUPS[0] * T])
            kt00 = qkpool.tile([128, GROUPS[0] * T], FP8, tag="kt0")
            nc.sync.dma_start(kt00[:], kT.ap()[0, :, 0:GROUPS[0] * T])
            mask_t = cst.tile([T, T], F32, tag="mask")
            nc.sync.dma_start(mask_t[:], mask.ap())
            id_t = cst.tile([T, T], BF16, tag="ident")
            nc.sync.dma_start(id_t[:], ident.ap())
            bds = []
            for p in range(2):
                bd = cst.tile([128, 128], BF16, tag=f"bd{p}")
                nc.vector.memset(bd[:], 0.0)
                bds.append(bd)

            for p in range(2):
                # ---- raw logits: 256 chunk matmuls, K=128 each
                lg_ps = lgpool.tile([T, 512], F32)
                ci = 0
                for gi, gch in enumerate(GROUPS):
                    if p == 0 and gi == 0:
                        qt, kt = qt00, kt00
                    else:
                        qt = qkpool.tile([128, gch * T], FP8, tag=f"qt{gi}")
                        nc.sync.dma_start(qt[:], qT.ap()[p, :, ci * T:(ci + gch) * T])
                        kt = qkpool.tile([128, gch * T], FP8, tag=f"kt{gi}")
                        nc.sync.dma_start(kt[:], kT.ap()[p, :, ci * T:(ci + gch) * T])
                    for cc in range(gch):
                        nc.tensor.matmul(lg_ps[:, 0:T],
                                         qt[:, cc * T:(cc + 1) * T],
                                         kt[:, cc * T:(cc + 1) * T],
                                         start=(ci == 0), stop=(ci == NCH - 1))
                        ci += 1

                # ---- softmax over s; the 1/sqrt(D) scale rides the exp
                lg = smpool.tile([T, T], F32, tag="lg")
                nc.vector.tensor_add(lg[:], lg_ps[:, 0:T], mask_t[:])
                mx = smpool.tile([T, 1], F32, tag="mx")
                nc.vector.reduce_max(mx[:], lg[:], axis=mybir.AxisListType.X, negate=True)
                mx_s = smpool.tile([T, 1], F32, tag="mxs")
                nc.vector.tensor_scalar(mx_s[:], mx[:], SCALE, None,
                                        op0=mybir.AluOpType.mult)
                e = smpool.tile([T, T], F32, tag="e")
                sm_acc = smpool.tile([T, 1], F32, tag="smacc")
                nc.scalar.activation(e[:], lg[:], mybir.ActivationFunctionType.Exp,
                                     bias=mx_s[:], scale=SCALE, accum_out=sm_acc[:])
                rc = smpool.tile([T, 1], F32, tag="rc")
                nc.vector.reciprocal(rc[:], sm_acc[:])
                att = smpool.tile([T, T], BF16, tag="att")
                nc.vector.tensor_scalar(att[:], e[:], rc[:], None,
                                        op0=mybir.AluOpType.mult)

                # ---- attT placed block-diagonally for K=128 att@v
                ps_t = pstpool.tile([T, 1024], BF16)
                nc.tensor.transpose(ps_t[:, 0:T], att[:], id_t[:])
                nc.vector.tensor_copy(bds[p][0:T, 0:T], ps_t[:, 0:T])
                nc.vector.tensor_copy(bds[p][T:128, T:128], ps_t[:, 0:T])

                # ---- y = att @ v, both channels per matmul; the last pieces
                # shrink so the drain->out tail after the final v lands is short
                pieces = [4096, 4096, 4096, 2048, 2048]
                off = 0
                for pw in pieces:
                    vt = vpool.tile([128, 4096], BF16)
                    nc.sync.dma_start(vt[0:128, 0:pw], vs.ap()[p, :, off:off + pw])
                    yst = ypool.tile([128, 4096], BF16)
                    for j in range(pw // 512):
                        psy = psypool.tile([128, 512], F32)
                        nc.tensor.matmul(psy[:], bds[p][:],
                                         vt[:, j * 512:(j + 1) * 512],
                                         start=True, stop=True)
                        dst = yst[:, j * 512:(j + 1) * 512]
                        if j % 2 == 0:
                            nc.vector.tensor_copy(dst, psy[:])
                        else:
                            nc.scalar.copy(dst, psy[:])
                    nc.gpsimd.dma_start(ys.ap()[p, :, off:off + pw], yst[0:128, 0:pw])
                    off += pw
    nc.compile()
    return nc


def _get(name):
    if name not in _BUILD_CACHE:
        _BUILD_CACHE[name] = {"l1": _build_l1, "l2": _build_l2, "l3": _build_l3}[name]()
    return _BUILD_CACHE[name]


# ---------------- host-side packing ----------------

def _build_planes(imgs_chw, ncopies):
    """imgs_chw: [N, 16, H, W] f32 -> [N, 16*ncopies+1, HP*WP] bf16.

    Row r = kx*16 + c holds xpad[c, :, :] shifted left by kx; last row ones."""
    N = imgs_chw.shape[0]
    xpad = np.zeros((N, C, HP, WP), np.float32)
    xpad[:, :, 1:H + 1, 1:W + 1] = np.asarray(imgs_chw, np.float32)
    flat = xpad.reshape(N, C, HP * WP)
    npl = 16 * ncopies + 1
    p = np.empty((N, npl, HP * WP), np.float32)
    for kx in range(ncopies):
        p[:, kx * 16:(kx + 1) * 16, 0:HP * WP - kx] = flat[:, :, kx:]
        if kx:
            p[:, kx * 16:(kx + 1) * 16, HP * WP - kx:] = 0.0
    p[:, npl - 1] = 1.0
    return p.astype(NPBF16)


def _build_w_l1(ws, bs):
    """-> (w0 [48,48], w1 [49,48], w2 [48,48]); row r = kx*16+c, bias on w1."""
    out = []
    for ky in range(3):
        m = np.zeros((49, 48), np.float32)
        for j, w in enumerate(ws):
            for kx in range(3):
                m[kx * 16:(kx + 1) * 16, j * 16:(j + 1) * 16] = w[:, :, ky, kx].T
        if ky == 1:
            for j, b in enumerate(bs):
                m[48, j * 16:(j + 1) * 16] = b
            out.append(m.astype(NPBF16))
        else:
            out.append(m[0:48].astype(NPBF16))
    return out


def _build_w_l3(w, b):
    """-> (w3 [49,48], wa [33,48], wb [16,48]): col block = ky*16 + o.

    w3 rows kx0,kx1,kx2,ones (49-row planes); wa rows kx0,kx1,ones (33-row
    planes); wb rows = kx2 taps, read from the kx0 planes at +2."""
    w3 = np.zeros((49, 48), np.float32)
    wa = np.zeros((33, 48), np.float32)
    wb = np.zeros((16, 48), np.float32)
    for ky in range(3):
        for kx in range(3):
            w3[kx * 16:(kx + 1) * 16, ky * 16:(ky + 1) * 16] = w[:, :, ky, kx].T
        wa[0:16, ky * 16:(ky + 1) * 16] = w[:, :, ky, 0].T
        wa[16:32, ky * 16:(ky + 1) * 16] = w[:, :, ky, 1].T
        wb[:, ky * 16:(ky + 1) * 16] = w[:, :, ky, 2].T
    w3[48, 16:32] = b  # bias rides ky=1's ones row
    wa[32, 16:32] = b
    return w3.astype(NPBF16), wa.astype(NPBF16), wb.astype(NPBF16)


def _unpack_conv(o, nout):
    """[N, 128 x, 128*nout (y-major, ch-minor)] -> [N, nout, HW]."""
    N = o.shape[0]
    return np.ascontiguousarray(
        o.reshape(N, 128, 128, nout).transpose(0, 3, 2, 1)).reshape(N, nout, HW)


def _pack_chunkT(a):
    """[2 ch, T, HW] -> chunk-major transposed [128, 256*T] (dtype-preserving)."""
    a = np.ascontiguousarray(a.transpose(0, 2, 1)).reshape(D, T)
    return np.ascontiguousarray(a.reshape(256, 128, T).transpose(1, 0, 2)).reshape(128, 256 * T)


# ---------------- top level ----------------

def kernel(x, wq, bq, wk, bk, wv, bv, wo, bo):
    x, wq, bq, wk, bk, wv, bv, wo, bo = (
        np.asarray(a, np.float32) for a in (x, wq, bq, wk, bk, wv, bv, wo, bo))
    ximg = x.reshape(IMGS, C, H, W)
    cores = list(range(NCORES))

    # ---- L1: q/k/v convs, image-sharded
    w0, w1, w2 = _build_w_l1([wq, wk, wv], [bq, bk, bv])
    wpk1 = np.zeros((49, 144), NPBF16)
    wpk1[0:48, 0:48] = w0
    wpk1[0:49, 48:96] = w1
    wpk1[0:48, 96:144] = w2
    in_maps = [{"planes": _build_planes(ximg[c * IPC:(c + 1) * IPC], 3),
                "wpk": wpk1} for c in cores]
    res1 = bass_utils.run_bass_kernel_spmd(_get("l1"), in_maps, core_ids=cores)

    qk_all = np.empty((B, 32, T, HW), NPFP8)   # q: 0-15, k: 16-31
    v_all = np.empty((B, 16, T, HW), NPBF16)
    for c in cores:
        qk = _unpack_conv(res1.results[c]["out_qk"], 32)
        v = _unpack_conv(res1.results[c]["out_v"], 16)
        b0 = (c * IPC) // T
        t0 = (c * IPC) % T
        qk_all[b0, :, t0:t0 + IPC] = qk.transpose(1, 0, 2)
        v_all[b0, :, t0:t0 + IPC] = v.transpose(1, 0, 2)

    # ---- L2: attention, head-sharded (2 (b,head) pairs per core)
    mask = np.triu(np.full((T, T), -30000.0, np.float32), 1)
    ident = np.eye(T, dtype=NPBF16)
    in_maps = []
    for c in cores:
        qTs = np.empty((2, 128, 256 * T), NPFP8)
        kTs = np.empty((2, 128, 256 * T), NPFP8)
        vss = np.empty((2, 128, HW), NPBF16)
        for p in range(2):
            pi = 2 * c + p
            b, h = divmod(pi, NH)
            qTs[p] = _pack_chunkT(qk_all[b, 2 * h:2 * h + 2])
            kTs[p] = _pack_chunkT(qk_all[b, 16 + 2 * h:16 + 2 * h + 2])
            vss[p] = v_all[b, 2 * h:2 * h + 2].reshape(128, HW)
        in_maps.append({"qT": qTs, "kT": kTs, "vs": vss, "mask": mask, "ident": ident})
    res2 = bass_utils.run_bass_kernel_spmd(_get("l2"), in_maps, core_ids=cores)

    y_all = np.empty((B, 16, T, HW), NPBF16)
    for c in cores:
        ys = res2.results[c]["ys"]  # [2, 128 (c,t), HW]
        for p in range(2):
            pi = 2 * c + p
            b, h = divmod(pi, NH)
            y_all[b, 2 * h:2 * h + 2] = ys[p].reshape(2, T, HW)

    # ---- L3: output conv, image-sharded (last N3S3 images use 49-row planes)
    yimg = np.ascontiguousarray(
        y_all.astype(np.float32).transpose(0, 2, 1, 3)).reshape(IMGS, 16, H, W)
    w3, wa, wb = _build_w_l3(wo, bo)
    wpk3 = np.zeros((49, 144), NPBF16)
    wpk3[0:49, 0:48] = w3
    wpk3[0:33, 48:96] = wa
    wpk3[0:16, 96:144] = wb
    in_maps = [{"planes6": _build_planes(yimg[c * IPC:(c + 1) * IPC - N3S3], 2),
                "planes3": _build_planes(yimg[(c + 1) * IPC - N3S3:(c + 1) * IPC], 3),
                "wpk": wpk3} for c in cores]
    res3 = bass_utils.run_bass_kernel_spmd(_get("l3"), in_maps, core_ids=cores)

    out = np.concatenate([_unpack_conv(res3.results[c]["out"], 16).astype(np.float32)
                          for c in cores])
    return np.ascontiguousarray(out.reshape(B, T, O, H, W))


# revision 58
# speedup vs baseline: 2.5803x; 1.0311x over previous
"""Trainium2 Bass kernel for nn_CNNT_enhanced_denoising_runtime_53704271069472.

Computes, distributed across 8 NeuronCores:
    q/k/v = conv3x3(x, w?, b?)          (image-sharded: B*T=128 imgs, 16/core)
    att   = causal-softmax(q @ k^T / sqrt(D)) per (batch, head)
    y     = att @ v                      (head-sharded: 16 (b,head) pairs, 2/core)
    out   = conv3x3(y, wo, bo)           (image-sharded)

Three SPMD launches with host-side resharding between them.

Convs use the matmul orientation lhsT = image planes (stationary, M = 128
pixels of one image row), rhs = weights (moving, N = out-channels) — the
per-matmul cost is the small N, not the pixel count. L1 planes are 49 rows
(16 ch x 3 kx-shifts + ones row for bias); the 3 ky taps are
PSUM-accumulating matmuls with a free-dim row offset. L3 mixes 33-row
planes (2 kx copies, 6 matmuls/row; the kx=2 tap reads the kx=0 rows at
+2) with a few 49-row images to balance the PE and DMA floors.

q/k travel as fp8e4m3 (they only feed the softmax logits, where the
relative error is crushed by the tiny logit scale); v/y stay bf16. The
1/sqrt(D) logit scale is folded into the softmax exp (fp8 can't hold
pre-scaled q).

Attention per (batch, head) pair: logits = 256 K=128 chunk matmuls
(lhsT = qT chunk [128d, 64t], rhs = kT chunk [128d, 64s]) accumulated in
PSUM; row softmax; att transposed on the PE and placed block-diagonally in
a [128,128] lhsT so att@v contracts both head-channels in single K=128
matmuls. Host pre-transposes q/k into chunk-major layout.

Scheduling details: first input tiles are chunked so the PE starts ~3us in;
outputs ship in split DMAs so stages free early and the tail is short;
per-image drains alternate DVE/Act; input DMAs issue on SP, output DMAs on
the Pool SWDGE path (final slivers on idle HWDGE engines).
"""
import sys
import numpy as np

sys.path.insert(0, "/opt/trn_rl_repo")

import ml_dtypes  # noqa: E402
import concourse.bacc as bacc  # noqa: E402
import concourse.tile as tile  # noqa: E402
import concourse.bass as bass  # noqa: E402
from concourse import mybir, bass_utils  # noqa: E402

BF16 = mybir.dt.bfloat16
FP8 = mybir.dt.float8e4
F32 = mybir.dt.float32
NPBF16 = ml_dtypes.bfloat16
NPFP8 = ml_dtypes.float8_e4m3

B, T, C, H, W, O = 2, 64, 16, 128, 128, 16
HP, WP = H + 2, W + 2
HW = H * W
NH, HC = 8, 2
D = HC * HW
SCALE = float(1.0 / np.sqrt(np.float32(D)))
NCORES = 8
IMGS = B * T
IPC = IMGS // NCORES  # images per core

_BUILD_CACHE = {}


# ---------------- device programs ----------------

def _build_l1():
    """q/k/v convs. Out: q,k as fp8 [x, y*32+ch], v as bf16 [x, y*16+ch]."""
    nc = bacc.Bacc("TRN2", target_bir_lowering=False, debug=False)
    planes = nc.dram_tensor("planes", (IPC, 49, HP * WP), BF16, kind="ExternalInput")
    wpk = nc.dram_tensor("wpk", (49, 144), BF16, kind="ExternalInput")
    out_qk = nc.dram_tensor("out_qk", (IPC, 128, 128 * 32), FP8, kind="ExternalOutput")
    out_v = nc.dram_tensor("out_v", (IPC, 128, 128 * 16), BF16, kind="ExternalOutput")

    with tile.TileContext(nc) as tc:
        with tc.tile_pool(name="w", bufs=1) as wpool, \
             tc.tile_pool(name="pl", bufs=3) as plpool, \
             tc.tile_pool(name="pl0", bufs=1) as pl0pool, \
             tc.tile_pool(name="st", bufs=3) as stpool, \
             tc.tile_pool(name="ps", bufs=8, space="PSUM") as pspool:
            # image 0's planes ship in three chunks, smallest first, so the
            # PE starts as early as possible
            pt_a = pl0pool.tile([49, 34 * WP], BF16, tag="pt0a")
            nc.sync.dma_start(pt_a[:], planes.ap()[0, :, 0:34 * WP])
            wt = wpool.tile([49, 144], BF16, tag="wpk")
            nc.sync.dma_start(wt[:], wpk.ap())
            pt_b = pl0pool.tile([49, 66 * WP], BF16, tag="pt0b")
            nc.sync.dma_start(pt_b[:], planes.ap()[0, :, 32 * WP:98 * WP])
            pt_c = pl0pool.tile([49, 34 * WP], BF16, tag="pt0c")
            nc.sync.dma_start(pt_c[:], planes.ap()[0, :, 96 * WP:130 * WP])

            for img in range(IPC):
                if img == 0:

                    def lhsT(y, ky, rows):
                        if y < 32:
                            return pt_a[0:rows, (y + ky) * WP:(y + ky) * WP + 128]
                        if y < 96:
                            return pt_b[0:rows, (y - 32 + ky) * WP:(y - 32 + ky) * WP + 128]
                        return pt_c[0:rows, (y - 96 + ky) * WP:(y - 96 + ky) * WP + 128]
                else:
                    pt = plpool.tile([49, HP * WP], BF16)
                    nc.sync.dma_start(pt[:], planes.ap()[img])

                    def lhsT(y, ky, rows, pt=pt):
                        return pt[0:rows, (y + ky) * WP:(y + ky) * WP + 128]

                st_qk = stpool.tile([128, 128 * 32], FP8, tag="stqk")
                st_v = stpool.tile([128, 128 * 16], BF16, tag="stv")
                for g in range(13):
                    n = min(10, 128 - g * 10)
                    ps = pspool.tile([128, 512], F32)
                    for j in range(n):
                        y = g * 10 + j
                        pv = ps[:, j * 48:(j + 1) * 48]
                        for ky in range(3):
                            rows = 49 if ky == 1 else 48
                            nc.tensor.matmul(pv, lhsT(y, ky, rows),
                                             wt[0:rows, ky * 48:(ky + 1) * 48],
                                             start=(ky == 0), stop=(ky == 2))
                    src = ps[:, 0:n * 48].rearrange("p (y c) -> p y c", c=48)
                    dqk = st_qk[:, g * 320:(g * 10 + n) * 32].rearrange(
                        "p (y c) -> p y c", c=32)
                    dv = st_v[:, g * 160:(g * 10 + n) * 16].rearrange(
                        "p (y c) -> p y c", c=16)
                    if g % 2 == 0:
                        nc.vector.tensor_copy(dqk, src[:, :, 0:32])
                        nc.scalar.copy(dv, src[:, :, 32:48])
                    else:
                        nc.scalar.copy(dqk, src[:, :, 0:32])
                        nc.vector.tensor_copy(dv, src[:, :, 32:48])
                    if g == 5:  # first 60 rows' columns can ship early
                        nc.gpsimd.dma_start(out_qk.ap()[img, :, 0:1920], st_qk[:, 0:1920])
                        nc.gpsimd.dma_start(out_v.ap()[img, :, 0:960], st_v[:, 0:960])
                    if img == IPC - 1 and g == 9:  # shrink the tail DMA
                        nc.gpsimd.dma_start(out_qk.ap()[img, :, 1920:3200],
                                            st_qk[:, 1920:3200])
                        nc.gpsimd.dma_start(out_v.ap()[img, :, 960:1600],
                                            st_v[:, 960:1600])
                if img == IPC - 1:
                    # final slivers over HWDGE on otherwise-idle engines
                    nc.sync.dma_start(out_qk.ap()[img, :, 3200:4096], st_qk[:, 3200:4096])
                    nc.scalar.dma_start(out_v.ap()[img, :, 1600:2048], st_v[:, 1600:2048])
                else:
                    nc.gpsimd.dma_start(out_qk.ap()[img, :, 1920:4096], st_qk[:, 1920:4096])
                    nc.gpsimd.dma_start(out_v.ap()[img, :, 960:2048], st_v[:, 960:2048])
    nc.compile()
    return nc


S3_IMGS = (3, 7, 11, 14)  # images using 49-row planes / 3 matmuls (DMA-heavy,
N3S3 = len(S3_IMGS)   # PE-light), spread out to keep both devices fed


def _build_l3():
    """Output conv. First N3S3 images use 49-row planes (3 matmuls/row), the
    rest 33-row planes (6 matmuls/row, kx=2 tap reads kx=0 rows at +2) —
    the mix balances the PE and DMA floors."""
    nc = bacc.Bacc("TRN2", target_bir_lowering=False, debug=False)
    planes6 = nc.dram_tensor("planes6", (IPC - N3S3, 33, HP * WP), BF16,
                             kind="ExternalInput")
    planes3 = nc.dram_tensor("planes3", (N3S3, 49, HP * WP), BF16, kind="ExternalInput")
    # packed weights: cols 0:48 = w3 (ky blocks), 48:96 = wa, 96:144 = wb
    wpk = nc.dram_tensor("wpk", (49, 144), BF16, kind="ExternalInput")
    out = nc.dram_tensor("out", (IPC, 128, 128 * 16), BF16, kind="ExternalOutput")

    with tile.TileContext(nc) as tc:
        with tc.tile_pool(name="w", bufs=1) as wpool, \
             tc.tile_pool(name="pl", bufs=3) as plpool, \
             tc.tile_pool(name="pl0", bufs=1) as pl0pool, \
             tc.tile_pool(name="st", bufs=3) as stpool, \
             tc.tile_pool(name="ps", bufs=4, space="PSUM") as pspool:
            pt_a = pl0pool.tile([33, 34 * WP], BF16, tag="pt0a")
            nc.sync.dma_start(pt_a[:], planes6.ap()[0, :, 0:34 * WP])
            wt = wpool.tile([49, 144], BF16, tag="wpk")
            nc.sync.dma_start(wt[:], wpk.ap())
            pt_b = pl0pool.tile([33, 66 * WP], BF16, tag="pt0b")
            nc.sync.dma_start(pt_b[:], planes6.ap()[0, :, 32 * WP:98 * WP])
            pt_c = pl0pool.tile([33, 34 * WP], BF16, tag="pt0c")
            nc.sync.dma_start(pt_c[:], planes6.ap()[0, :, 96 * WP:130 * WP])

            i3 = i6 = 0
            for img in range(IPC):
                s3 = img in S3_IMGS
                if img == 0:
                    i6 += 1

                    def win(y, ky, rows, dx=0):
                        if y < 32:
                            return pt_a[0:rows, (y + ky) * WP + dx:(y + ky) * WP + dx + 128]
                        if y < 96:
                            b = (y - 32 + ky) * WP + dx
                            return pt_b[0:rows, b:b + 128]
                        b = (y - 96 + ky) * WP + dx
                        return pt_c[0:rows, b:b + 128]
                else:
                    pt = plpool.tile([49, HP * WP], BF16)
                    if s3:
                        nc.sync.dma_start(pt[:], planes3.ap()[i3])
                        i3 += 1
                    else:
                        nc.sync.dma_start(pt[0:33, :], planes6.ap()[i6])
                        i6 += 1

                    def win(y, ky, rows, dx=0, pt=pt):
                        b = (y + ky) * WP + dx
                        return pt[0:rows, b:b + 128]

                stage = stpool.tile([128, 128 * 16], BF16)
                # 64 y-rows per 2-bank PSUM tile (32 per bank, dense 16-col)
                for g in range(2):
                    ps = pspool.tile([128, 1024], F32)
                    for j in range(64):
                        y = g * 64 + j
                        pv = ps[:, j * 16:(j + 1) * 16]
                        for ky in range(3):
                            if s3:
                                rows = 49 if ky == 1 else 48
                                nc.tensor.matmul(pv, win(y, ky, rows),
                                                 wt[0:rows, ky * 16:(ky + 1) * 16],
                                                 start=(ky == 0), stop=(ky == 2))
                            else:
                                rows = 33 if ky == 1 else 32
                                nc.tensor.matmul(pv, win(y, ky, rows),
                                                 wt[0:rows, 48 + ky * 16:48 + (ky + 1) * 16],
                                                 start=(ky == 0), stop=False)
                                nc.tensor.matmul(pv, win(y, ky, 16, dx=2),
                                                 wt[0:16, 96 + ky * 16:96 + (ky + 1) * 16],
                                                 start=False, stop=(ky == 2))
                    if img == IPC - 1 and g == 1:
                        # split the last drain across both engines, ship the
                        # slivers over idle HWDGE paths
                        nc.vector.tensor_copy(stage[:, 1024:1536], ps[:, 0:512])
                        nc.scalar.copy(stage[:, 1536:2048], ps[:, 512:1024])
                        nc.sync.dma_start(out.ap()[img, :, 1024:1536],
                                          stage[:, 1024:1536])
                        nc.scalar.dma_start(out.ap()[img, :, 1536:2048],
                                            stage[:, 1536:2048])
                    else:
                        eng = nc.vector if g % 2 == 0 else nc.scalar
                        dst = stage[:, g * 1024:(g + 1) * 1024]
                        if eng is nc.vector:
                            eng.tensor_copy(dst, ps[:])
                        else:
                            eng.copy(dst, ps[:])
                        if g == 0:
                            nc.gpsimd.dma_start(out.ap()[img, :, 0:1024],
                                                stage[:, 0:1024])
                if img != IPC - 1:
                    nc.gpsimd.dma_start(out.ap()[img, :, 1024:2048], stage[:, 1024:2048])
    nc.compile()
    return nc


def _build_l2():
    nc = bacc.Bacc("TRN2", target_bir_lowering=False, debug=False)
    qT = nc.dram_tensor("qT", (2, 128, D // 2), FP8, kind="ExternalInput")
    kT = nc.dram_tensor("kT", (2, 128, D // 2), FP8, kind="ExternalInput")
    vs = nc.dram_tensor("vs", (2, 128, HW), BF16, kind="ExternalInput")
    mask = nc.dram_tensor("mask", (T, T), F32, kind="ExternalInput")
    ident = nc.dram_tensor("ident", (T, T), BF16, kind="ExternalInput")
    ys = nc.dram_tensor("ys", (2, 128, HW), BF16, kind="ExternalOutput")

    NCH = 256                        # K=128 contraction chunks per pair
    GROUPS = [16, 48, 64, 64, 64]    # chunks per DMA group (small head chunk)

    with tile.TileContext(nc) as tc:
        with tc.tile_pool(name="cst", bufs=1) as cst, \
             tc.tile_pool(name="qk", bufs=2) as qkpool, \
             tc.tile_pool(name="sm", bufs=2) as smpool, \
             tc.tile_pool(name="v", bufs=3) as vpool, \
             tc.tile_pool(name="yst", bufs=3) as ypool, \
             tc.tile_pool(name="pslg", bufs=2, space="PSUM") as lgpool, \
             tc.tile_pool(name="pst", bufs=1, space="PSUM") as pstpool, \
             tc.tile_pool(name="psy", bufs=4, space="PSUM") as psypool:
            # first q/k chunks ship before everything else so the PE can start
            qt00 = qkpool.tile([128, GROUPS[0] * T], FP8, tag="qt0")
            nc.sync.dma_start(qt00[:], qT.ap()[0, :, 0:GROUPS[0] * T])
            kt00 = qkpool.tile([128, GROUPS[0] * T], FP8, tag="kt0")
            nc.sync.dma_start(kt00[:], kT.ap()[0, :, 0:GROUPS[0] * T])
            mask_t = cst.tile([T, T], F32, tag="mask")
            nc.sync.dma_start(mask_t[:], mask.ap())
            id_t = cst.tile([T, T], BF16, tag="ident")
            nc.sync.dma_start(id_t[:], ident.ap())
            bds = []
            for p in range(2):
                bd = cst.tile([128, 128], BF16, tag=f"bd{p}")
                nc.vector.memset(bd[:], 0.0)
                bds.append(bd)

            for p in range(2):
                # ---- raw logits: 256 chunk matmuls, K=128 each
                lg_ps = lgpool.tile([T, 512], F32)
                ci = 0
                for gi, gch in enumerate(GROUPS):
                    if p == 0 and gi == 0:
                        qt, kt = qt00, kt00
                    else:
                        qt = qkpool.tile([128, gch * T], FP8, tag=f"qt{gi}")
                        nc.sync.dma_start(qt[:], qT.ap()[p, :, ci * T:(ci + gch) * T])
                        kt = qkpool.tile([128, gch * T], FP8, tag=f"kt{gi}")
                        nc.sync.dma_start(kt[:], kT.ap()[p, :, ci * T:(ci + gch) * T])
                    for cc in range(gch):
                        nc.tensor.matmul(lg_ps[:, 0:T],
                                         qt[:, cc * T:(cc + 1) * T],
                                         kt[:, cc * T:(cc + 1) * T],
                                         start=(ci == 0), stop=(ci == NCH - 1))
                        ci += 1

                # ---- softmax over s; the 1/sqrt(D) scale rides the exp
                lg = smpool.tile([T, T], F32, tag="lg")
                nc.vector.tensor_add(lg[:], lg_ps[:, 0:T], mask_t[:])
                mx = smpool.tile([T, 1], F32, tag="mx")
                nc.vector.reduce_max(mx[:], lg[:], axis=mybir.AxisListType.X, negate=True)
                mx_s = smpool.tile([T, 1], F32, tag="mxs")
                nc.vector.tensor_scalar(mx_s[:], mx[:], SCALE, None,
                                        op0=mybir.AluOpType.mult)
                e = smpool.tile([T, T], F32, tag="e")
                sm_acc = smpool.tile([T, 1], F32, tag="smacc")
                nc.scalar.activation(e[:], lg[:], mybir.ActivationFunctionType.Exp,
                                     bias=mx_s[:], scale=SCALE, accum_out=sm_acc[:])
                rc = smpool.tile([T, 1], F32, tag="rc")
                nc.vector.reciprocal(rc[:], sm_acc[:])
                att = smpool.tile([T, T], BF16, tag="att")
                nc.vector.tensor_scalar(att[:], e[:], rc[:], None,
                                        op0=mybir.AluOpType.mult)

                # ---- attT placed block-diagonally for K=128 att@v
                ps_t = pstpool.tile([T, 1024], BF16)
                nc.tensor.transpose(ps_t[:, 0:T], att[:], id_t[:])
                nc.vector.tensor_copy(bds[p][0:T, 0:T], ps_t[:, 0:T])
                nc.vector.tensor_copy(bds[p][T:128, T:128], ps_t[:, 0:T])

                # ---- y = att @ v, both channels per matmul; the last pieces
                # shrink so the drain->out tail after the final v lands is short
                pieces = [4096, 4096, 4096, 2048, 1024, 1024]
                off = 0
                for pw in pieces:
                    vt = vpool.tile([128, 4096], BF16)
                    nc.sync.dma_start(vt[0:128, 0:pw], vs.ap()[p, :, off:off + pw])
                    yst = ypool.tile([128, 4096], BF16)
                    for j in range(pw // 512):
                        psy = psypool.tile([128, 512], F32)
                        nc.tensor.matmul(psy[:], bds[p][:],
                                         vt[:, j * 512:(j + 1) * 512],
                                         start=True, stop=True)
                        dst = yst[:, j * 512:(j + 1) * 512]
                        if j % 2 == 0:
                            nc.vector.tensor_copy(dst, psy[:])
                        else:
                            nc.scalar.copy(dst, psy[:])
                    nc.gpsimd.dma_start(ys.ap()[p, :, off:off + pw], yst[0:128, 0:pw])
                    off += pw
    nc.compile()
    return nc


def _get(name):
    if name not in _BUILD_CACHE:
        _BUILD_CACHE[name] = {"l1": _build_l1, "l2": _build_l2, "l3": _build_l3}[name]()
    return _BUILD_CACHE[name]


# ---------------- host-side packing ----------------

def _build_planes(imgs_chw, ncopies):
    """imgs_chw: [N, 16, H, W] f32 -> [N, 16*ncopies+1, HP*WP] bf16.

    Row r = kx*16 + c holds xpad[c, :, :] shifted left by kx; last row ones."""
    N = imgs_chw.shape[0]
    xpad = np.zeros((N, C, HP, WP), np.float32)
    xpad[:, :, 1:H + 1, 1:W + 1] = np.asarray(imgs_chw, np.float32)
    flat = xpad.reshape(N, C, HP * WP)
    npl = 16 * ncopies + 1
    p = np.empty((N, npl, HP * WP), np.float32)
    for kx in range(ncopies):
        p[:, kx * 16:(kx + 1) * 16, 0:HP * WP - kx] = flat[:, :, kx:]
        if kx:
            p[:, kx * 16:(kx + 1) * 16, HP * WP - kx:] = 0.0
    p[:, npl - 1] = 1.0
    return p.astype(NPBF16)


def _build_w_l1(ws, bs):
    """-> (w0 [48,48], w1 [49,48], w2 [48,48]); row r = kx*16+c, bias on w1."""
    out = []
    for ky in range(3):
        m = np.zeros((49, 48), np.float32)
        for j, w in enumerate(ws):
            for kx in range(3):
                m[kx * 16:(kx + 1) * 16, j * 16:(j + 1) * 16] = w[:, :, ky, kx].T
        if ky == 1:
            for j, b in enumerate(bs):
                m[48, j * 16:(j + 1) * 16] = b
            out.append(m.astype(NPBF16))
        else:
            out.append(m[0:48].astype(NPBF16))
    return out


def _build_w_l3(w, b):
    """-> (w3 [49,48], wa [33,48], wb [16,48]): col block = ky*16 + o.

    w3 rows kx0,kx1,kx2,ones (49-row planes); wa rows kx0,kx1,ones (33-row
    planes); wb rows = kx2 taps, read from the kx0 planes at +2."""
    w3 = np.zeros((49, 48), np.float32)
    wa = np.zeros((33, 48), np.float32)
    wb = np.zeros((16, 48), np.float32)
    for ky in range(3):
        for kx in range(3):
            w3[kx * 16:(kx + 1) * 16, ky * 16:(ky + 1) * 16] = w[:, :, ky, kx].T
        wa[0:16, ky * 16:(ky + 1) * 16] = w[:, :, ky, 0].T
        wa[16:32, ky * 16:(ky + 1) * 16] = w[:, :, ky, 1].T
        wb[:, ky * 16:(ky + 1) * 16] = w[:, :, ky, 2].T
    w3[48, 16:32] = b  # bias rides ky=1's ones row
    wa[32, 16:32] = b
    return w3.astype(NPBF16), wa.astype(NPBF16), wb.astype(NPBF16)


def _unpack_conv(o, nout):
    """[N, 128 x, 128*nout (y-major, ch-minor)] -> [N, nout, HW]."""
    N = o.shape[0]
    return np.ascontiguousarray(
        o.reshape(N, 128, 128, nout).transpose(0, 3, 2, 1)).reshape(N, nout, HW)


def _pack_chunkT(a):
    """[2 ch, T, HW] -> chunk-major transposed [128, 256*T] (dtype-preserving)."""
    a = np.ascontiguousarray(a.transpose(0, 2, 1)).reshape(D, T)
    return np.ascontiguousarray(a.reshape(256, 128, T).transpose(1, 0, 2)).reshape(128, 256 * T)


# ---------------- top level ----------------

def kernel(x, wq, bq, wk, bk, wv, bv, wo, bo):
    x, wq, bq, wk, bk, wv, bv, wo, bo = (
        np.asarray(a, np.float32) for a in (x, wq, bq, wk, bk, wv, bv, wo, bo))
    ximg = x.reshape(IMGS, C, H, W)
    cores = list(range(NCORES))

    # ---- L1: q/k/v convs, image-sharded
    w0, w1, w2 = _build_w_l1([wq, wk, wv], [bq, bk, bv])
    wpk1 = np.zeros((49, 144), NPBF16)
    wpk1[0:48, 0:48] = w0
    wpk1[0:49, 48:96] = w1
    wpk1[0:48, 96:144] = w2
    in_maps = [{"planes": _build_planes(ximg[c * IPC:(c + 1) * IPC], 3),
                "wpk": wpk1} for c in cores]
    res1 = bass_utils.run_bass_kernel_spmd(_get("l1"), in_maps, core_ids=cores)

    qk_all = np.empty((B, 32, T, HW), NPFP8)   # q: 0-15, k: 16-31
    v_all = np.empty((B, 16, T, HW), NPBF16)
    for c in cores:
        qk = _unpack_conv(res1.results[c]["out_qk"], 32)
        v = _unpack_conv(res1.results[c]["out_v"], 16)
        b0 = (c * IPC) // T
        t0 = (c * IPC) % T
        qk_all[b0, :, t0:t0 + IPC] = qk.transpose(1, 0, 2)
        v_all[b0, :, t0:t0 + IPC] = v.transpose(1, 0, 2)

    # ---- L2: attention, head-sharded (2 (b,head) pairs per core)
    mask = np.triu(np.full((T, T), -30000.0, np.float32), 1)
    ident = np.eye(T, dtype=NPBF16)
    in_maps = []
    for c in cores:
        qTs = np.empty((2, 128, 256 * T), NPFP8)
        kTs = np.empty((2, 128, 256 * T), NPFP8)
        vss = np.empty((2, 128, HW), NPBF16)
        for p in range(2):
            pi = 2 * c + p
            b, h = divmod(pi, NH)
            qTs[p] = _pack_chunkT(qk_all[b, 2 * h:2 * h + 2])
            kTs[p] = _pack_chunkT(qk_all[b, 16 + 2 * h:16 + 2 * h + 2])
            vss[p] = v_all[b, 2 * h:2 * h + 2].reshape(128, HW)
        in_maps.append({"qT": qTs, "kT": kTs, "vs": vss, "mask": mask, "ident": ident})
    res2 = bass_utils.run_bass_kernel_spmd(_get("l2"), in_maps, core_ids=cores)

    y_all = np.empty((B, 16, T, HW), NPBF16)
    for c in cores:
        ys = res2.results[c]["ys"]  # [2, 128 (c,t), HW]
        for p in range(2):
            pi = 2 * c + p
            b, h = divmod(pi, NH)
            y_all[b, 2 * h:2 * h + 2] = ys[p].reshape(2, T, HW)

    # ---- L3: output conv, image-sharded (last N3S3 images use 49-row planes)
    yimg = np.ascontiguousarray(
        y_all.astype(np.float32).transpose(0, 2, 1, 3)).reshape(IMGS, 16, H, W)
    w3, wa, wb = _build_w_l3(wo, bo)
    wpk3 = np.zeros((49, 144), NPBF16)
    wpk3[0:49, 0:48] = w3
    wpk3[0:33, 48:96] = wa
    wpk3[0:16, 96:144] = wb
    i6_idx = [i for i in range(IPC) if i not in S3_IMGS]
    in_maps = [{"planes6": _build_planes(yimg[[c * IPC + i for i in i6_idx]], 2),
                "planes3": _build_planes(yimg[[c * IPC + i for i in S3_IMGS]], 3),
                "wpk": wpk3} for c in cores]
    res3 = bass_utils.run_bass_kernel_spmd(_get("l3"), in_maps, core_ids=cores)

    out = np.concatenate([_unpack_conv(res3.results[c]["out"], 16).astype(np.float32)
                          for c in cores])
    return np.ascontiguousarray(out.reshape(B, T, O, H, W))
